# revision 24
# baseline (speedup 1.0000x reference)
"""MoE (8 experts, top-2) Trainium2 kernel.

Strategy (expert-parallel, per spec sharding hint):
  Launch A (router, data-parallel over tokens): each of the 8 cores computes
    router logits for its 1024-token shard on the PE (true fp32), then top-2
    expert ids via the DVE max8/max-index instructions.
  Host glue: build per-expert token gather lists from topk ids (pure data
    movement / indexing), gather + transpose activations per expert.
  Launch B (FFN, expert-parallel): core c runs expert c's MLP over the tokens
    routed to it:  yT = 0.5*(gelu(x@W1+b1)@W2+b2)^T,  float32r matmuls
    (1 cyc/row on PE), gelu fused into PSUM eviction on ACT.
  Host glue: scatter-add the two expert contributions per token (the top-2
    mean; 0.5 factor already applied on device).
"""

import os
import numpy as np

try:
    import concourse.bass as bass  # noqa: F401
except ImportError:  # pragma: no cover
    import sys

    sys.path.insert(0, "/opt/trn_rl_repo")

import concourse.bass as bass
import concourse.tile as tile
from concourse import bacc, mybir
from concourse.bass_utils import run_bass_kernel_spmd

NCORES = 8
B, N, E, H, X, TOPK = 8, 1024, 1024, 1536, 8, 2
T = B * N
TOK = T // NCORES  # tokens per core in the router launch

F32 = mybir.dt.float32
F32R = mybir.dt.float32r
U32 = mybir.dt.uint32
I32 = mybir.dt.int32
AF = mybir.ActivationFunctionType
BF16 = mybir.dt.bfloat16

ET = E // 128  # 8 k-tiles over embedding dim
HT = H // 128  # 12 tiles over hidden dim


def _pe_warmup(nc, pool, psum_pool, n_mm=50):
    """Issue dummy bf16 matmuls so the PE HAM clock-gate reaches 8/8 while
    the initial DMAs are still in flight (cold PE runs at half clock)."""
    wu = pool.tile([128, 128], BF16, tag="warmup")
    nc.vector.memset(wu[:], 0.0)
    pw = psum_pool.tile([128, 128], F32, tag="warmup_ps")
    for _ in range(n_mm):
        nc.tensor.matmul(pw[:], wu[:], wu[:], start=True, stop=True)

_program_cache = {}
LAST_RESULTS = {}


def _make_nc():
    return bacc.Bacc(
        "TRN2", target_bir_lowering=False, debug=False, num_devices=NCORES
    )


def _build_router():
    """Per core: logits^T[x, t] = (Wr.T @ x^T) for its 1024-token shard.

    Weight-stationary (Wr as lhsT, only 8 columns to load), tokens as the
    512-wide moving operand, true fp32. The [8, 512] logit stripes are
    PE-transposed back to [tok, 8] for the DVE top-2 instructions.
    """
    from concourse.masks import make_identity

    nc = _make_nc()
    xT = nc.dram_tensor("xT", [E, TOK], F32, kind="ExternalInput")
    Wr = nc.dram_tensor("Wr", [E, X], F32, kind="ExternalInput")
    br = nc.dram_tensor("br", [1, X], F32, kind="ExternalInput")
    topk = nc.dram_tensor("topk", [TOK, TOPK], I32, kind="ExternalOutput")

    NB = TOK // 512  # token stripes per core

    with tile.TileContext(nc) as tc:
        with (
            tc.tile_pool(name="const", bufs=1) as const,
            tc.tile_pool(name="xts", bufs=1) as xtp,
            tc.tile_pool(name="wk", bufs=6) as wk,
            tc.tile_pool(name="ps", bufs=2, space="PSUM") as ps,
            tc.tile_pool(name="pst", bufs=4, space="PSUM") as pst,
            tc.tile_pool(name="wups", bufs=1, space="PSUM") as wups,
        ):
            _pe_warmup(nc, wk, wups)
            Wr_sb = const.tile([128, ET, X], F32)
            nc.sync.dma_start(Wr_sb[:], Wr.rearrange("(a p) x -> p a x", p=128))
            br_sb = const.tile([1, X], F32)
            nc.sync.dma_start(br_sb[:], br[0:1, :])
            ones = const.tile([1, 512], F32)
            nc.vector.memset(ones[:], 1.0)
            id8 = const.tile([8, 8], F32)
            make_identity(nc, id8[:])

            xT_view = xT.rearrange("(a p) t -> p a t", p=128)
            # all input stripes prefetched upfront, per-k-tile granularity so
            # the first matmul starts after 256KB; outputs go on the gpsimd
            # (SWDGE) queue so they never delay input loads
            xts_all = {}
            for nb in range(NB):
                for a in range(ET):
                    t = xtp.tile(
                        [128, 512], F32, tag=f"xts{nb}_{a}", name=f"xts_{nb}_{a}"
                    )
                    nc.sync.dma_start(
                        t[:], xT_view[:, a, nb * 512 : (nb + 1) * 512]
                    )
                    xts_all[nb, a] = t
            for nb in range(NB):
                xts = [xts_all[nb, a] for a in range(ET)]
                ps_t = ps.tile([8, 512], F32, tag="ps")
                for a in range(ET):
                    nc.tensor.matmul(
                        ps_t[:],
                        Wr_sb[:, a, :],
                        xts[a][:],
                        start=(a == 0),
                        stop=False,
                    )
                # bias add as a rank-1 matmul: br[1,8].T @ ones[1,512]
                nc.tensor.matmul(
                    ps_t[:], br_sb[:1, :], ones[:1, :], start=False, stop=True
                )
                lt = wk.tile([8, 512], F32, tag="lt")
                nc.vector.tensor_copy(lt[:], ps_t[:])
                for j in range(4):
                    ps_tr = pst.tile([128, 8], F32, tag="pst")
                    nc.tensor.transpose(
                        ps_tr[:], lt[:, j * 128 : (j + 1) * 128], id8[:]
                    )
                    logits = wk.tile([128, X], F32, tag="logits")
                    nc.vector.tensor_copy(logits[:], ps_tr[:])
                    mx = wk.tile([128, 8], F32, tag="mx")
                    mi = wk.tile([128, 8], U32, tag="mi")
                    nc.vector.max_with_indices(mx[:], mi[:], logits[:])
                    t0 = nb * 512 + j * 128
                    nc.gpsimd.dma_start(
                        topk[t0 : t0 + 128, :], mi[:, :TOPK].bitcast(I32)
                    )
    nc.compile()
    return nc


def _build_ffn(cap):
    assert cap % 8 == 0
    nc = _make_nc()
    xgT = nc.dram_tensor("xgT", [E, cap], F32R, kind="ExternalInput")
    W1 = nc.dram_tensor("W1", [E, H], F32R, kind="ExternalInput")
    b1 = nc.dram_tensor("b1", [H], F32, kind="ExternalInput")
    W2 = nc.dram_tensor("W2", [H, E], F32R, kind="ExternalInput")
    b2 = nc.dram_tensor("b2", [E], F32, kind="ExternalInput")
    yT = nc.dram_tensor("yT", [E, cap], F32, kind="ExternalOutput")

    # token chunking: near-equal chunks of 128-multiples, each <= 512 and
    # >= 256 so float32r matmuls stay at 1 cycle/row
    nch = -(-cap // 512)
    widths = [512] * nch
    slack = 512 * nch - cap
    i = nch - 1
    while slack > 0:
        d = min(slack, widths[i] - 256) if nch > 1 else slack
        d = (d // 8) * 8
        if d <= 0:
            i = nch - 1 if i == 0 else i - 1
            continue
        widths[i] -= d
        slack -= d
        i = nch - 1 if i == 0 else i - 1
    assert sum(widths) == cap and all(w >= 256 for w in widths), widths
    widths.sort()  # narrowest first: cheapest chunk while weights stream in
    chunks = []
    c0 = 0
    for cw in widths:
        chunks.append((c0, cw))
        c0 += cw

    with tile.TileContext(nc) as tc:
        with (
            tc.tile_pool(name="wconst", bufs=1) as wconst,
            tc.tile_pool(name="xg", bufs=1) as xgp,
            tc.tile_pool(name="g", bufs=2) as gp,
            tc.tile_pool(name="y", bufs=3) as yp,
            tc.tile_pool(name="ps1", bufs=2, space="PSUM") as ps1,
            tc.tile_pool(name="ps2", bufs=2, space="PSUM") as ps2,
            tc.tile_pool(name="wups", bufs=1, space="PSUM") as wups,
        ):
            _pe_warmup(nc, yp, wups)
            # ALL activation chunks ride the gpsimd (SWDGE) DMA queue,
            # emitted upfront, so they are never stuck behind the 12MB of
            # weights on the sync (HWDGE) queue. Chunk 0 is split per k-tile
            # so the first matmul only waits for 1/8 of a chunk.
            xg_view = xgT.rearrange("(a p) t -> p a t", p=128)
            xg0_c0, xg0_cw = chunks[0]
            xg0_parts = [
                wconst.tile([128, xg0_cw], F32R, tag=f"xg0_{a}", name=f"xg0_{a}")
                for a in range(ET)
            ]
            for a in range(ET):
                nc.gpsimd.dma_start(
                    xg0_parts[a][:], xg_view[:, a, xg0_c0 : xg0_c0 + xg0_cw]
                )
            xg_tiles = {}
            for ci, (c0, cw) in enumerate(chunks):
                if ci == 0:
                    continue
                t = xgp.tile([128, ET, cw], F32R, tag="xg", name=f"xg_{ci}")
                nc.gpsimd.dma_start(t[:], xg_view[:, :, c0 : c0 + cw])
                xg_tiles[ci] = t

            b1_sb = wconst.tile([128, HT], F32)
            nc.sync.dma_start(b1_sb[:], b1.rearrange("(h p) -> p h", p=128))
            b2_sb = wconst.tile([128, ET], F32)
            nc.sync.dma_start(b2_sb[:], b2.rearrange("(e p) -> p e", p=128))
            b2h_sb = wconst.tile([128, ET], F32)
            nc.scalar.mul(b2h_sb[:], b2_sb[:], 0.5)

            # weights resident in SBUF, split by hidden tile and DMA'd in
            # consumption order, alternating between the two HWDGE queues
            # (sync + scalar) to double weight delivery rate at the start.
            # Safe on the ACT queue: weight loads never wait on semaphores.
            w1_sb = []
            w1_view = W1.rearrange("(a p) (h j) -> h p a j", p=128, j=128)
            for h in range(HT):
                t = wconst.tile([128, ET, 128], F32R, tag=f"w1_{h}")
                eng = nc.sync if h % 2 == 0 else nc.scalar
                eng.dma_start(t[:], w1_view[h])
                w1_sb.append(t)
            w2_sb = []
            w2_view = W2.rearrange("(h p) e -> h p e", p=128)
            for h in range(HT):
                t = wconst.tile([128, E], F32R, tag=f"w2_{h}")
                eng = nc.sync if h % 2 == 1 else nc.scalar
                eng.dma_start(t[:], w2_view[h])
                w2_sb.append(t)

            def phase1(ci):
                c0, cw = chunks[ci]
                if ci == 0:
                    xg_slice = lambda a: xg0_parts[a][:]
                else:
                    xg_sb = xg_tiles[ci]
                    xg_slice = lambda a, xg_sb=xg_sb: xg_sb[:, a, :]
                g_sb = gp.tile([128, HT, cw], F32R, tag="g", name=f"g_{ci}")
                for h in range(HT):
                    pt = ps1.tile([128, cw], F32, tag="ps1", name=f"p1_{ci}_{h}")
                    for a in range(ET):
                        nc.tensor.matmul(
                            pt[:],
                            w1_sb[h][:, a, :],
                            xg_slice(a),
                            start=(a == 0),
                            stop=(a == ET - 1),
                        )
                    nc.scalar.activation(
                        g_sb[:, h, :], pt[:], AF.Gelu, bias=b1_sb[:, h : h + 1]
                    )
                return g_sb

            def phase2(ci, g_sb):
                c0, cw = chunks[ci]
                for eo in range(ET):
                    pt2 = ps2.tile([128, cw], F32, tag="ps2", name=f"p2_{ci}_{eo}")
                    for h in range(HT):
                        nc.tensor.matmul(
                            pt2[:],
                            w2_sb[h][:, eo * 128 : (eo + 1) * 128],
                            g_sb[:, h, :],
                            start=(h == 0),
                            stop=(h == HT - 1),
                        )
                    y_sb = yp.tile([128, cw], F32, tag="y", name=f"y_{ci}_{eo}")
                    # y = 0.5*psum + 0.5*b2  on the (idle) DVE, keeping ACT
                    # free for GELU and avoiding ACT LUT thrash
                    nc.vector.tensor_scalar(
                        y_sb[:],
                        pt2[:],
                        0.5,
                        b2h_sb[:, eo : eo + 1],
                        op0=mybir.AluOpType.mult,
                        op1=mybir.AluOpType.add,
                    )
                    nc.sync.dma_start(
                        yT[eo * 128 : (eo + 1) * 128, c0 : c0 + cw], y_sb[:]
                    )

            for ci in range(len(chunks)):
                phase2(ci, phase1(ci))
    nc.compile()
    return nc


def _get_program(key, builder, *args):
    if key not in _program_cache:
        _program_cache[key] = builder(*args)
    return _program_cache[key]


def _run(nc, in_maps):
    """run_bass_kernel_spmd with one retry for transient runtime errors."""
    try:
        return run_bass_kernel_spmd(nc, in_maps, list(range(NCORES)))
    except Exception:
        return run_bass_kernel_spmd(nc, in_maps, list(range(NCORES)))


def kernel(x, W_router, b_router, W1, b1, W2, b2):
    x = np.ascontiguousarray(np.asarray(x, dtype=np.float32))
    W_router = np.ascontiguousarray(np.asarray(W_router, dtype=np.float32))
    b_router = np.ascontiguousarray(np.asarray(b_router, dtype=np.float32))
    W1 = np.ascontiguousarray(np.asarray(W1, dtype=np.float32))
    b1 = np.ascontiguousarray(np.asarray(b1, dtype=np.float32))
    W2 = np.ascontiguousarray(np.asarray(W2, dtype=np.float32))
    b2 = np.ascontiguousarray(np.asarray(b2, dtype=np.float32))

    x_flat = x.reshape(T, E)
    xT = np.ascontiguousarray(x_flat.T)  # [E, T]
    br2 = b_router.reshape(1, X)

    # ---- launch A: router ----
    ncr = _get_program("router", _build_router)
    in_maps = [
        {"xT": np.ascontiguousarray(xT[:, c * TOK : (c + 1) * TOK]), "Wr": W_router, "br": br2}
        for c in range(NCORES)
    ]
    res_a = _run(ncr, in_maps)
    LAST_RESULTS["router"] = res_a
    topk = np.concatenate(
        [res_a.results[c]["topk"] for c in range(NCORES)], axis=0
    )  # [T, 2] int32

    # ---- host glue: per-expert gather lists ----
    sel = [np.flatnonzero((topk[:, 0] == c) | (topk[:, 1] == c)) for c in range(X)]
    counts = [len(s) for s in sel]
    cap = max(512, -(-max(counts) // 8) * 8)

    ncf = _get_program(("ffn", cap), _build_ffn, cap)
    in_maps = []
    for c in range(X):
        idx = np.zeros(cap, dtype=np.int64)
        idx[: counts[c]] = sel[c]
        xgT = np.ascontiguousarray(x_flat[idx].T)  # [E, cap]
        in_maps.append(
            {"xgT": xgT, "W1": W1[c], "b1": b1[c], "W2": W2[c], "b2": b2[c]}
        )
    res_b = _run(ncf, in_maps)
    LAST_RESULTS["ffn"] = res_b

    # ---- host glue: scatter-add the two 0.5-scaled expert outputs per token ----
    out = np.zeros((T, E), dtype=np.float32)
    for c in range(X):
        yT_c = res_b.results[c]["yT"]  # [E, cap]
        out[sel[c]] += yT_c[:, : counts[c]].T
    return out.reshape(B, N, E), topk.reshape(B, N, TOPK).astype(np.int32)


# revision 25
# speedup vs baseline: 1.0347x; 1.0347x over previous
"""MoE (8 experts, top-2) Trainium2 kernel.

Strategy (expert-parallel, per spec sharding hint):
  Launch A (router, data-parallel over tokens): each of the 8 cores computes
    router logits for its 1024-token shard on the PE (true fp32), then top-2
    expert ids via the DVE max8/max-index instructions.
  Host glue: build per-expert token gather lists from topk ids (pure data
    movement / indexing), gather + transpose activations per expert.
  Launch B (FFN, expert-parallel): core c runs expert c's MLP over the tokens
    routed to it:  yT = 0.5*(gelu(x@W1+b1)@W2+b2)^T,  float32r matmuls
    (1 cyc/row on PE), gelu fused into PSUM eviction on ACT.
  Host glue: scatter-add the two expert contributions per token (the top-2
    mean; 0.5 factor already applied on device).
"""

import os
import numpy as np

try:
    import concourse.bass as bass  # noqa: F401
except ImportError:  # pragma: no cover
    import sys

    sys.path.insert(0, "/opt/trn_rl_repo")

import concourse.bass as bass
import concourse.tile as tile
from concourse import bacc, mybir
from concourse.bass_utils import run_bass_kernel_spmd

NCORES = 8
B, N, E, H, X, TOPK = 8, 1024, 1024, 1536, 8, 2
T = B * N
TOK = T // NCORES  # tokens per core in the router launch

F32 = mybir.dt.float32
F32R = mybir.dt.float32r
U32 = mybir.dt.uint32
I32 = mybir.dt.int32
AF = mybir.ActivationFunctionType
BF16 = mybir.dt.bfloat16

ET = E // 128  # 8 k-tiles over embedding dim
HT = H // 128  # 12 tiles over hidden dim


def _pe_warmup(nc, pool, psum_pool, n_mm=50):
    """Issue dummy bf16 matmuls so the PE HAM clock-gate reaches 8/8 while
    the initial DMAs are still in flight (cold PE runs at half clock)."""
    wu = pool.tile([128, 128], BF16, tag="warmup")
    nc.vector.memset(wu[:], 0.0)
    pw = psum_pool.tile([128, 128], F32, tag="warmup_ps")
    for _ in range(n_mm):
        nc.tensor.matmul(pw[:], wu[:], wu[:], start=True, stop=True)

_program_cache = {}
LAST_RESULTS = {}


def _make_nc():
    return bacc.Bacc(
        "TRN2", target_bir_lowering=False, debug=False, num_devices=NCORES
    )


def _build_router():
    """Per core: logits^T[x, t] = (Wr.T @ x^T) for its 1024-token shard.

    Weight-stationary (Wr as lhsT, only 8 columns to load), tokens as the
    512-wide moving operand, true fp32. The [8, 512] logit stripes are
    PE-transposed back to [tok, 8] for the DVE top-2 instructions.
    """
    from concourse.masks import make_identity

    nc = _make_nc()
    xT = nc.dram_tensor("xT", [E, TOK], F32, kind="ExternalInput")
    Wr = nc.dram_tensor("Wr", [E, X], F32, kind="ExternalInput")
    br = nc.dram_tensor("br", [1, X], F32, kind="ExternalInput")
    topk = nc.dram_tensor("topk", [TOK, TOPK], I32, kind="ExternalOutput")

    NB = TOK // 512  # token stripes per core

    with tile.TileContext(nc) as tc:
        with (
            tc.tile_pool(name="const", bufs=1) as const,
            tc.tile_pool(name="xts", bufs=1) as xtp,
            tc.tile_pool(name="wk", bufs=6) as wk,
            tc.tile_pool(name="ps", bufs=2, space="PSUM") as ps,
            tc.tile_pool(name="pst", bufs=4, space="PSUM") as pst,
            tc.tile_pool(name="wups", bufs=1, space="PSUM") as wups,
        ):
            _pe_warmup(nc, wk, wups)
            Wr_sb = const.tile([128, ET, X], F32)
            nc.sync.dma_start(Wr_sb[:], Wr.rearrange("(a p) x -> p a x", p=128))
            br_sb = const.tile([1, X], F32)
            nc.sync.dma_start(br_sb[:], br[0:1, :])
            ones = const.tile([1, 512], F32)
            nc.vector.memset(ones[:], 1.0)
            id8 = const.tile([8, 8], F32)
            make_identity(nc, id8[:])

            xT_view = xT.rearrange("(a p) t -> p a t", p=128)
            # all input stripes prefetched upfront, per-k-tile granularity so
            # the first matmul starts after 256KB; outputs go on the gpsimd
            # (SWDGE) queue so they never delay input loads
            xts_all = {}
            for nb in range(NB):
                for a in range(ET):
                    t = xtp.tile(
                        [128, 512], F32, tag=f"xts{nb}_{a}", name=f"xts_{nb}_{a}"
                    )
                    nc.sync.dma_start(
                        t[:], xT_view[:, a, nb * 512 : (nb + 1) * 512]
                    )
                    xts_all[nb, a] = t
            for nb in range(NB):
                xts = [xts_all[nb, a] for a in range(ET)]
                ps_t = ps.tile([8, 512], F32, tag="ps")
                for a in range(ET):
                    nc.tensor.matmul(
                        ps_t[:],
                        Wr_sb[:, a, :],
                        xts[a][:],
                        start=(a == 0),
                        stop=False,
                    )
                # bias add as a rank-1 matmul: br[1,8].T @ ones[1,512]
                nc.tensor.matmul(
                    ps_t[:], br_sb[:1, :], ones[:1, :], start=False, stop=True
                )
                lt = wk.tile([8, 512], F32, tag="lt")
                nc.vector.tensor_copy(lt[:], ps_t[:])
                for j in range(4):
                    ps_tr = pst.tile([128, 8], F32, tag="pst")
                    nc.tensor.transpose(
                        ps_tr[:], lt[:, j * 128 : (j + 1) * 128], id8[:]
                    )
                    logits = wk.tile([128, X], F32, tag="logits")
                    nc.vector.tensor_copy(logits[:], ps_tr[:])
                    mx = wk.tile([128, 8], F32, tag="mx")
                    mi = wk.tile([128, 8], U32, tag="mi")
                    nc.vector.max_with_indices(mx[:], mi[:], logits[:])
                    t0 = nb * 512 + j * 128
                    nc.gpsimd.dma_start(
                        topk[t0 : t0 + 128, :], mi[:, :TOPK].bitcast(I32)
                    )
    nc.compile()
    return nc


def _build_ffn(cap):
    assert cap % 8 == 0
    nc = _make_nc()
    xgT = nc.dram_tensor("xgT", [E, cap], F32R, kind="ExternalInput")
    W1 = nc.dram_tensor("W1", [E, H], F32R, kind="ExternalInput")
    b1 = nc.dram_tensor("b1", [H], F32, kind="ExternalInput")
    W2 = nc.dram_tensor("W2", [H, E], F32R, kind="ExternalInput")
    b2 = nc.dram_tensor("b2", [E], F32, kind="ExternalInput")
    yT = nc.dram_tensor("yT", [E, cap], F32, kind="ExternalOutput")

    # token chunking: near-equal chunks of 128-multiples, each <= 512 and
    # >= 256 so float32r matmuls stay at 1 cycle/row
    nch = -(-cap // 512)
    widths = [512] * nch
    slack = 512 * nch - cap
    i = nch - 1
    while slack > 0:
        d = min(slack, widths[i] - 256) if nch > 1 else slack
        d = (d // 8) * 8
        if d <= 0:
            i = nch - 1 if i == 0 else i - 1
            continue
        widths[i] -= d
        slack -= d
        i = nch - 1 if i == 0 else i - 1
    assert sum(widths) == cap and all(w >= 256 for w in widths), widths
    widths.sort()  # narrowest first: cheapest chunk while weights stream in
    chunks = []
    c0 = 0
    for cw in widths:
        chunks.append((c0, cw))
        c0 += cw

    with tile.TileContext(nc) as tc:
        with (
            tc.tile_pool(name="wconst", bufs=1) as wconst,
            tc.tile_pool(name="xg", bufs=1) as xgp,
            tc.tile_pool(name="g", bufs=2) as gp,
            tc.tile_pool(name="y", bufs=3) as yp,
            tc.tile_pool(name="ps1", bufs=2, space="PSUM") as ps1,
            tc.tile_pool(name="ps2", bufs=2, space="PSUM") as ps2,
            tc.tile_pool(name="wups", bufs=1, space="PSUM") as wups,
        ):
            _pe_warmup(nc, yp, wups)
            # ALL activation chunks ride the gpsimd (SWDGE) DMA queue,
            # emitted upfront, so they are never stuck behind the 12MB of
            # weights on the sync (HWDGE) queue. Chunk 0 is split per k-tile
            # so the first matmul only waits for 1/8 of a chunk.
            xg_view = xgT.rearrange("(a p) t -> p a t", p=128)
            xg0_c0, xg0_cw = chunks[0]
            xg0_parts = [
                wconst.tile([128, xg0_cw], F32R, tag=f"xg0_{a}", name=f"xg0_{a}")
                for a in range(ET)
            ]
            for a in range(ET):
                nc.gpsimd.dma_start(
                    xg0_parts[a][:], xg_view[:, a, xg0_c0 : xg0_c0 + xg0_cw]
                )
            xg_tiles = {}
            for ci, (c0, cw) in enumerate(chunks):
                if ci == 0:
                    continue
                t = xgp.tile([128, ET, cw], F32R, tag="xg", name=f"xg_{ci}")
                nc.gpsimd.dma_start(t[:], xg_view[:, :, c0 : c0 + cw])
                xg_tiles[ci] = t

            b1_sb = wconst.tile([128, HT], F32)
            nc.sync.dma_start(b1_sb[:], b1.rearrange("(h p) -> p h", p=128))
            b2_sb = wconst.tile([128, ET], F32)
            nc.sync.dma_start(b2_sb[:], b2.rearrange("(e p) -> p e", p=128))
            b2h_sb = wconst.tile([128, ET], F32)
            nc.scalar.mul(b2h_sb[:], b2_sb[:], 0.5)

            # weights resident in SBUF, split by hidden tile (DMA'd in the
            # order the PE consumes them: W1 h-tiles, then W2 h-tiles)
            w1_sb = []
            w1_view = W1.rearrange("(a p) (h j) -> h p a j", p=128, j=128)
            for h in range(HT):
                t = wconst.tile([128, ET, 128], F32R, tag=f"w1_{h}")
                nc.sync.dma_start(t[:], w1_view[h])
                w1_sb.append(t)
            w2_sb = []
            w2_view = W2.rearrange("(h p) e -> h p e", p=128)
            for h in range(HT):
                t = wconst.tile([128, E], F32R, tag=f"w2_{h}")
                nc.sync.dma_start(t[:], w2_view[h])
                w2_sb.append(t)

            def phase1(ci):
                c0, cw = chunks[ci]
                if ci == 0:
                    xg_slice = lambda a: xg0_parts[a][:]
                else:
                    xg_sb = xg_tiles[ci]
                    xg_slice = lambda a, xg_sb=xg_sb: xg_sb[:, a, :]
                g_sb = gp.tile([128, HT, cw], F32R, tag="g", name=f"g_{ci}")
                for h in range(HT):
                    pt = ps1.tile([128, cw], F32, tag="ps1", name=f"p1_{ci}_{h}")
                    for a in range(ET):
                        nc.tensor.matmul(
                            pt[:],
                            w1_sb[h][:, a, :],
                            xg_slice(a),
                            start=(a == 0),
                            stop=(a == ET - 1),
                        )
                    nc.scalar.activation(
                        g_sb[:, h, :], pt[:], AF.Gelu, bias=b1_sb[:, h : h + 1]
                    )
                return g_sb

            def phase2(ci, g_sb):
                c0, cw = chunks[ci]
                for eo in range(ET):
                    pt2 = ps2.tile([128, cw], F32, tag="ps2", name=f"p2_{ci}_{eo}")
                    for h in range(HT):
                        nc.tensor.matmul(
                            pt2[:],
                            w2_sb[h][:, eo * 128 : (eo + 1) * 128],
                            g_sb[:, h, :],
                            start=(h == 0),
                            stop=(h == HT - 1),
                        )
                    y_sb = yp.tile([128, cw], F32, tag="y", name=f"y_{ci}_{eo}")
                    # y = 0.5*psum + 0.5*b2  on the (idle) DVE, keeping ACT
                    # free for GELU and avoiding ACT LUT thrash
                    nc.vector.tensor_scalar(
                        y_sb[:],
                        pt2[:],
                        0.5,
                        b2h_sb[:, eo : eo + 1],
                        op0=mybir.AluOpType.mult,
                        op1=mybir.AluOpType.add,
                    )
                    nc.sync.dma_start(
                        yT[eo * 128 : (eo + 1) * 128, c0 : c0 + cw], y_sb[:]
                    )

            for ci in range(len(chunks)):
                phase2(ci, phase1(ci))
    nc.compile()
    return nc


def _get_program(key, builder, *args):
    if key not in _program_cache:
        _program_cache[key] = builder(*args)
    return _program_cache[key]


def _run(nc, in_maps):
    """run_bass_kernel_spmd with one retry for transient runtime errors."""
    try:
        return run_bass_kernel_spmd(nc, in_maps, list(range(NCORES)))
    except Exception:
        return run_bass_kernel_spmd(nc, in_maps, list(range(NCORES)))


def kernel(x, W_router, b_router, W1, b1, W2, b2):
    x = np.ascontiguousarray(np.asarray(x, dtype=np.float32))
    W_router = np.ascontiguousarray(np.asarray(W_router, dtype=np.float32))
    b_router = np.ascontiguousarray(np.asarray(b_router, dtype=np.float32))
    W1 = np.ascontiguousarray(np.asarray(W1, dtype=np.float32))
    b1 = np.ascontiguousarray(np.asarray(b1, dtype=np.float32))
    W2 = np.ascontiguousarray(np.asarray(W2, dtype=np.float32))
    b2 = np.ascontiguousarray(np.asarray(b2, dtype=np.float32))

    x_flat = x.reshape(T, E)
    xT = np.ascontiguousarray(x_flat.T)  # [E, T]
    br2 = b_router.reshape(1, X)

    # ---- launch A: router ----
    ncr = _get_program("router", _build_router)
    in_maps = [
        {"xT": np.ascontiguousarray(xT[:, c * TOK : (c + 1) * TOK]), "Wr": W_router, "br": br2}
        for c in range(NCORES)
    ]
    res_a = _run(ncr, in_maps)
    LAST_RESULTS["router"] = res_a
    topk = np.concatenate(
        [res_a.results[c]["topk"] for c in range(NCORES)], axis=0
    )  # [T, 2] int32

    # ---- host glue: per-expert gather lists ----
    sel = [np.flatnonzero((topk[:, 0] == c) | (topk[:, 1] == c)) for c in range(X)]
    counts = [len(s) for s in sel]
    cap = max(512, -(-max(counts) // 8) * 8)

    ncf = _get_program(("ffn", cap), _build_ffn, cap)
    in_maps = []
    for c in range(X):
        idx = np.zeros(cap, dtype=np.int64)
        idx[: counts[c]] = sel[c]
        xgT = np.ascontiguousarray(x_flat[idx].T)  # [E, cap]
        in_maps.append(
            {"xgT": xgT, "W1": W1[c], "b1": b1[c], "W2": W2[c], "b2": b2[c]}
        )
    res_b = _run(ncf, in_maps)
    LAST_RESULTS["ffn"] = res_b

    # ---- host glue: scatter-add the two 0.5-scaled expert outputs per token ----
    out = np.zeros((T, E), dtype=np.float32)
    for c in range(X):
        yT_c = res_b.results[c]["yT"]  # [E, cap]
        out[sel[c]] += yT_c[:, : counts[c]].T
    return out.reshape(B, N, E), topk.reshape(B, N, TOPK).astype(np.int32)


# revision 26
# speedup vs baseline: 1.0445x; 1.0094x over previous
"""MoE (8 experts, top-2) Trainium2 kernel.

Strategy (expert-parallel, per spec sharding hint):
  Launch A (router, data-parallel over tokens): each of the 8 cores computes
    router logits for its 1024-token shard on the PE (true fp32), then top-2
    expert ids via the DVE max8/max-index instructions.
  Host glue: build per-expert token gather lists from topk ids (pure data
    movement / indexing), gather + transpose activations per expert.
  Launch B (FFN, expert-parallel): core c runs expert c's MLP over the tokens
    routed to it:  yT = 0.5*(gelu(x@W1+b1)@W2+b2)^T,  float32r matmuls
    (1 cyc/row on PE), gelu fused into PSUM eviction on ACT.
  Host glue: scatter-add the two expert contributions per token (the top-2
    mean; 0.5 factor already applied on device).
"""

import os
import numpy as np

try:
    import concourse.bass as bass  # noqa: F401
except ImportError:  # pragma: no cover
    import sys

    sys.path.insert(0, "/opt/trn_rl_repo")

import concourse.bass as bass
import concourse.tile as tile
from concourse import bacc, mybir
from concourse.bass_utils import run_bass_kernel_spmd

NCORES = 8
B, N, E, H, X, TOPK = 8, 1024, 1024, 1536, 8, 2
T = B * N
TOK = T // NCORES  # tokens per core in the router launch

F32 = mybir.dt.float32
F32R = mybir.dt.float32r
U32 = mybir.dt.uint32
I32 = mybir.dt.int32
AF = mybir.ActivationFunctionType
BF16 = mybir.dt.bfloat16

ET = E // 128  # 8 k-tiles over embedding dim
HT = H // 128  # 12 tiles over hidden dim


def _pe_warmup(nc, pool, psum_pool, n_mm=50):
    """Issue dummy bf16 matmuls so the PE HAM clock-gate reaches 8/8 while
    the initial DMAs are still in flight (cold PE runs at half clock)."""
    wu = pool.tile([128, 128], BF16, tag="warmup")
    nc.vector.memset(wu[:], 0.0)
    pw = psum_pool.tile([128, 128], F32, tag="warmup_ps")
    for _ in range(n_mm):
        nc.tensor.matmul(pw[:], wu[:], wu[:], start=True, stop=True)

_program_cache = {}
LAST_RESULTS = {}


def _make_nc():
    return bacc.Bacc(
        "TRN2", target_bir_lowering=False, debug=False, num_devices=NCORES
    )


def _build_router():
    """Per core: logits^T[x, t] = (Wr.T @ x^T) for its 1024-token shard.

    Weight-stationary (Wr as lhsT, only 8 columns to load), tokens as the
    512-wide moving operand, true fp32. The [8, 512] logit stripes are
    PE-transposed back to [tok, 8] for the DVE top-2 instructions.
    """
    from concourse.masks import make_identity

    nc = _make_nc()
    xT = nc.dram_tensor("xT", [E, TOK], F32, kind="ExternalInput")
    Wr = nc.dram_tensor("Wr", [E, X], F32, kind="ExternalInput")
    br = nc.dram_tensor("br", [1, X], F32, kind="ExternalInput")
    topk = nc.dram_tensor("topk", [TOK, TOPK], I32, kind="ExternalOutput")

    NB = TOK // 512  # token stripes per core

    with tile.TileContext(nc) as tc:
        with (
            tc.tile_pool(name="const", bufs=1) as const,
            tc.tile_pool(name="xts", bufs=1) as xtp,
            tc.tile_pool(name="wk", bufs=6) as wk,
            tc.tile_pool(name="ps", bufs=2, space="PSUM") as ps,
            tc.tile_pool(name="pst", bufs=4, space="PSUM") as pst,
            tc.tile_pool(name="wups", bufs=1, space="PSUM") as wups,
        ):
            _pe_warmup(nc, wk, wups)
            Wr_sb = const.tile([128, ET, X], F32)
            nc.sync.dma_start(Wr_sb[:], Wr.rearrange("(a p) x -> p a x", p=128))
            br_sb = const.tile([1, X], F32)
            nc.sync.dma_start(br_sb[:], br[0:1, :])
            ones = const.tile([1, 512], F32)
            nc.vector.memset(ones[:], 1.0)
            id8 = const.tile([8, 8], F32)
            make_identity(nc, id8[:])

            xT_view = xT.rearrange("(a p) t -> p a t", p=128)
            # all input stripes prefetched upfront, per-k-tile granularity so
            # the first matmul starts after 256KB; outputs go on the gpsimd
            # (SWDGE) queue so they never delay input loads
            xts_all = {}
            for nb in range(NB):
                for a in range(ET):
                    t = xtp.tile(
                        [128, 512], F32, tag=f"xts{nb}_{a}", name=f"xts_{nb}_{a}"
                    )
                    nc.sync.dma_start(
                        t[:], xT_view[:, a, nb * 512 : (nb + 1) * 512]
                    )
                    xts_all[nb, a] = t
            for nb in range(NB):
                xts = [xts_all[nb, a] for a in range(ET)]
                ps_t = ps.tile([8, 512], F32, tag="ps")
                for a in range(ET):
                    nc.tensor.matmul(
                        ps_t[:],
                        Wr_sb[:, a, :],
                        xts[a][:],
                        start=(a == 0),
                        stop=False,
                    )
                # bias add as a rank-1 matmul: br[1,8].T @ ones[1,512]
                nc.tensor.matmul(
                    ps_t[:], br_sb[:1, :], ones[:1, :], start=False, stop=True
                )
                lt = wk.tile([8, 512], F32, tag="lt")
                nc.vector.tensor_copy(lt[:], ps_t[:])
                for j in range(4):
                    ps_tr = pst.tile([128, 8], F32, tag="pst")
                    nc.tensor.transpose(
                        ps_tr[:], lt[:, j * 128 : (j + 1) * 128], id8[:]
                    )
                    logits = wk.tile([128, X], F32, tag="logits")
                    nc.vector.tensor_copy(logits[:], ps_tr[:])
                    mx = wk.tile([128, 8], F32, tag="mx")
                    mi = wk.tile([128, 8], U32, tag="mi")
                    nc.vector.max_with_indices(mx[:], mi[:], logits[:])
                    t0 = nb * 512 + j * 128
                    nc.gpsimd.dma_start(
                        topk[t0 : t0 + 128, :], mi[:, :TOPK].bitcast(I32)
                    )
    nc.compile()
    return nc


def _build_ffn(cap):
    assert cap % 8 == 0
    nc = _make_nc()
    xgT = nc.dram_tensor("xgT", [E, cap], F32R, kind="ExternalInput")
    W1 = nc.dram_tensor("W1", [E, H], F32R, kind="ExternalInput")
    b1 = nc.dram_tensor("b1", [H], F32, kind="ExternalInput")
    W2 = nc.dram_tensor("W2", [H, E], F32R, kind="ExternalInput")
    b2 = nc.dram_tensor("b2", [E], F32, kind="ExternalInput")
    yT = nc.dram_tensor("yT", [E, cap], F32, kind="ExternalOutput")

    # token chunking: near-equal chunks of 128-multiples, each <= 512 and
    # >= 256 so float32r matmuls stay at 1 cycle/row
    nch = -(-cap // 512)
    widths = [512] * nch
    slack = 512 * nch - cap
    i = nch - 1
    while slack > 0:
        d = min(slack, widths[i] - 256) if nch > 1 else slack
        d = (d // 8) * 8
        if d <= 0:
            i = nch - 1 if i == 0 else i - 1
            continue
        widths[i] -= d
        slack -= d
        i = nch - 1 if i == 0 else i - 1
    assert sum(widths) == cap and all(w >= 256 for w in widths), widths
    widths.sort()  # narrowest first: cheapest chunk while weights stream in
    chunks = []
    c0 = 0
    for cw in widths:
        chunks.append((c0, cw))
        c0 += cw

    with tile.TileContext(nc) as tc:
        with (
            tc.tile_pool(name="wconst", bufs=1) as wconst,
            tc.tile_pool(name="xg", bufs=1) as xgp,
            tc.tile_pool(name="g", bufs=2) as gp,
            tc.tile_pool(name="y", bufs=3) as yp,
            tc.tile_pool(name="ps1", bufs=3, space="PSUM") as ps1,
            tc.tile_pool(name="ps2", bufs=3, space="PSUM") as ps2,
            tc.tile_pool(name="wups", bufs=1, space="PSUM") as wups,
        ):
            _pe_warmup(nc, yp, wups)
            # ALL activation chunks ride the gpsimd (SWDGE) DMA queue,
            # emitted upfront, so they are never stuck behind the 12MB of
            # weights on the sync (HWDGE) queue. Chunk 0 is split per k-tile
            # so the first matmul only waits for 1/8 of a chunk.
            xg_view = xgT.rearrange("(a p) t -> p a t", p=128)
            xg0_c0, xg0_cw = chunks[0]
            xg0_parts = [
                wconst.tile([128, xg0_cw], F32R, tag=f"xg0_{a}", name=f"xg0_{a}")
                for a in range(ET)
            ]
            for a in range(ET):
                nc.gpsimd.dma_start(
                    xg0_parts[a][:], xg_view[:, a, xg0_c0 : xg0_c0 + xg0_cw]
                )
            xg_tiles = {}
            for ci, (c0, cw) in enumerate(chunks):
                if ci == 0:
                    continue
                t = xgp.tile([128, ET, cw], F32R, tag="xg", name=f"xg_{ci}")
                nc.gpsimd.dma_start(t[:], xg_view[:, :, c0 : c0 + cw])
                xg_tiles[ci] = t

            b1_sb = wconst.tile([128, HT], F32)
            nc.sync.dma_start(b1_sb[:], b1.rearrange("(h p) -> p h", p=128))
            b2_sb = wconst.tile([128, ET], F32)
            nc.sync.dma_start(b2_sb[:], b2.rearrange("(e p) -> p e", p=128))
            b2h_sb = wconst.tile([128, ET], F32)
            nc.scalar.mul(b2h_sb[:], b2_sb[:], 0.5)

            # weights resident in SBUF, split by hidden tile (DMA'd in the
            # order the PE consumes them: W1 h-tiles, then W2 h-tiles)
            w1_sb = []
            w1_view = W1.rearrange("(a p) (h j) -> h p a j", p=128, j=128)
            for h in range(HT):
                t = wconst.tile([128, ET, 128], F32R, tag=f"w1_{h}")
                nc.sync.dma_start(t[:], w1_view[h])
                w1_sb.append(t)
            w2_sb = []
            w2_view = W2.rearrange("(h p) e -> h p e", p=128)
            for h in range(HT):
                t = wconst.tile([128, E], F32R, tag=f"w2_{h}")
                nc.sync.dma_start(t[:], w2_view[h])
                w2_sb.append(t)

            def phase1(ci):
                c0, cw = chunks[ci]
                if ci == 0:
                    xg_slice = lambda a: xg0_parts[a][:]
                else:
                    xg_sb = xg_tiles[ci]
                    xg_slice = lambda a, xg_sb=xg_sb: xg_sb[:, a, :]
                g_sb = gp.tile([128, HT, cw], F32R, tag="g", name=f"g_{ci}")
                for h in range(HT):
                    pt = ps1.tile([128, cw], F32, tag="ps1", name=f"p1_{ci}_{h}")
                    for a in range(ET):
                        nc.tensor.matmul(
                            pt[:],
                            w1_sb[h][:, a, :],
                            xg_slice(a),
                            start=(a == 0),
                            stop=(a == ET - 1),
                        )
                    nc.scalar.activation(
                        g_sb[:, h, :], pt[:], AF.Gelu, bias=b1_sb[:, h : h + 1]
                    )
                return g_sb

            def phase2(ci, g_sb):
                c0, cw = chunks[ci]
                for eo in range(ET):
                    pt2 = ps2.tile([128, cw], F32, tag="ps2", name=f"p2_{ci}_{eo}")
                    for h in range(HT):
                        nc.tensor.matmul(
                            pt2[:],
                            w2_sb[h][:, eo * 128 : (eo + 1) * 128],
                            g_sb[:, h, :],
                            start=(h == 0),
                            stop=(h == HT - 1),
                        )
                    y_sb = yp.tile([128, cw], F32, tag="y", name=f"y_{ci}_{eo}")
                    # y = 0.5*psum + 0.5*b2  on the (idle) DVE, keeping ACT
                    # free for GELU and avoiding ACT LUT thrash
                    nc.vector.tensor_scalar(
                        y_sb[:],
                        pt2[:],
                        0.5,
                        b2h_sb[:, eo : eo + 1],
                        op0=mybir.AluOpType.mult,
                        op1=mybir.AluOpType.add,
                    )
                    nc.sync.dma_start(
                        yT[eo * 128 : (eo + 1) * 128, c0 : c0 + cw], y_sb[:]
                    )

            for ci in range(len(chunks)):
                phase2(ci, phase1(ci))
    nc.compile()
    return nc


def _get_program(key, builder, *args):
    if key not in _program_cache:
        _program_cache[key] = builder(*args)
    return _program_cache[key]


def _run(nc, in_maps):
    """run_bass_kernel_spmd with one retry for transient runtime errors."""
    try:
        return run_bass_kernel_spmd(nc, in_maps, list(range(NCORES)))
    except Exception:
        return run_bass_kernel_spmd(nc, in_maps, list(range(NCORES)))


def kernel(x, W_router, b_router, W1, b1, W2, b2):
    x = np.ascontiguousarray(np.asarray(x, dtype=np.float32))
    W_router = np.ascontiguousarray(np.asarray(W_router, dtype=np.float32))
    b_router = np.ascontiguousarray(np.asarray(b_router, dtype=np.float32))
    W1 = np.ascontiguousarray(np.asarray(W1, dtype=np.float32))
    b1 = np.ascontiguousarray(np.asarray(b1, dtype=np.float32))
    W2 = np.ascontiguousarray(np.asarray(W2, dtype=np.float32))
    b2 = np.ascontiguousarray(np.asarray(b2, dtype=np.float32))

    x_flat = x.reshape(T, E)
    xT = np.ascontiguousarray(x_flat.T)  # [E, T]
    br2 = b_router.reshape(1, X)

    # ---- launch A: router ----
    ncr = _get_program("router", _build_router)
    in_maps = [
        {"xT": np.ascontiguousarray(xT[:, c * TOK : (c + 1) * TOK]), "Wr": W_router, "br": br2}
        for c in range(NCORES)
    ]
    res_a = _run(ncr, in_maps)
    LAST_RESULTS["router"] = res_a
    topk = np.concatenate(
        [res_a.results[c]["topk"] for c in range(NCORES)], axis=0
    )  # [T, 2] int32

    # ---- host glue: per-expert gather lists ----
    sel = [np.flatnonzero((topk[:, 0] == c) | (topk[:, 1] == c)) for c in range(X)]
    counts = [len(s) for s in sel]
    cap = max(512, -(-max(counts) // 8) * 8)

    ncf = _get_program(("ffn", cap), _build_ffn, cap)
    in_maps = []
    for c in range(X):
        idx = np.zeros(cap, dtype=np.int64)
        idx[: counts[c]] = sel[c]
        xgT = np.ascontiguousarray(x_flat[idx].T)  # [E, cap]
        in_maps.append(
            {"xgT": xgT, "W1": W1[c], "b1": b1[c], "W2": W2[c], "b2": b2[c]}
        )
    res_b = _run(ncf, in_maps)
    LAST_RESULTS["ffn"] = res_b

    # ---- host glue: scatter-add the two 0.5-scaled expert outputs per token ----
    out = np.zeros((T, E), dtype=np.float32)
    for c in range(X):
        yT_c = res_b.results[c]["yT"]  # [E, cap]
        out[sel[c]] += yT_c[:, : counts[c]].T
    return out.reshape(B, N, E), topk.reshape(B, N, TOPK).astype(np.int32)


# revision 27
# speedup vs baseline: 1.0569x; 1.0119x over previous
"""MoE (8 experts, top-2) Trainium2 kernel.

Strategy (expert-parallel, per spec sharding hint):
  Launch A (router, data-parallel over tokens): each of the 8 cores computes
    router logits for its 1024-token shard on the PE (true fp32), then top-2
    expert ids via the DVE max8/max-index instructions.
  Host glue: build per-expert token gather lists from topk ids (pure data
    movement / indexing), gather + transpose activations per expert.
  Launch B (FFN, expert-parallel): core c runs expert c's MLP over the tokens
    routed to it:  yT = 0.5*(gelu(x@W1+b1)@W2+b2)^T,  float32r matmuls
    (1 cyc/row on PE), gelu fused into PSUM eviction on ACT.
  Host glue: scatter-add the two expert contributions per token (the top-2
    mean; 0.5 factor already applied on device).
"""

import os
import numpy as np

try:
    import concourse.bass as bass  # noqa: F401
except ImportError:  # pragma: no cover
    import sys

    sys.path.insert(0, "/opt/trn_rl_repo")

import concourse.bass as bass
import concourse.tile as tile
from concourse import bacc, mybir
from concourse.bass_utils import run_bass_kernel_spmd

NCORES = 8
B, N, E, H, X, TOPK = 8, 1024, 1024, 1536, 8, 2
T = B * N
TOK = T // NCORES  # tokens per core in the router launch

F32 = mybir.dt.float32
F32R = mybir.dt.float32r
U32 = mybir.dt.uint32
I32 = mybir.dt.int32
AF = mybir.ActivationFunctionType
BF16 = mybir.dt.bfloat16

ET = E // 128  # 8 k-tiles over embedding dim
HT = H // 128  # 12 tiles over hidden dim


def _pe_warmup(nc, pool, psum_pool, n_mm=50):
    """Issue dummy bf16 matmuls so the PE HAM clock-gate reaches 8/8 while
    the initial DMAs are still in flight (cold PE runs at half clock)."""
    wu = pool.tile([128, 128], BF16, tag="warmup")
    nc.vector.memset(wu[:], 0.0)
    pw = psum_pool.tile([128, 128], F32, tag="warmup_ps")
    for _ in range(n_mm):
        nc.tensor.matmul(pw[:], wu[:], wu[:], start=True, stop=True)

_program_cache = {}
LAST_RESULTS = {}


def _make_nc():
    return bacc.Bacc(
        "TRN2", target_bir_lowering=False, debug=False, num_devices=NCORES
    )


def _build_router():
    """Per core: logits^T[x, t] = (Wr.T @ x^T) for its 1024-token shard.

    Weight-stationary (Wr as lhsT, only 8 columns to load), tokens as the
    512-wide moving operand, true fp32. The [8, 512] logit stripes are
    PE-transposed back to [tok, 8] for the DVE top-2 instructions.
    """
    from concourse.masks import make_identity

    nc = _make_nc()
    xT = nc.dram_tensor("xT", [E, TOK], F32, kind="ExternalInput")
    Wr = nc.dram_tensor("Wr", [E, X], F32, kind="ExternalInput")
    br = nc.dram_tensor("br", [1, X], F32, kind="ExternalInput")
    topk = nc.dram_tensor("topk", [TOK, TOPK], I32, kind="ExternalOutput")

    NB = TOK // 512  # token stripes per core

    with tile.TileContext(nc) as tc:
        with (
            tc.tile_pool(name="const", bufs=1) as const,
            tc.tile_pool(name="xts", bufs=1) as xtp,
            tc.tile_pool(name="wk", bufs=6) as wk,
            tc.tile_pool(name="ps", bufs=2, space="PSUM") as ps,
            tc.tile_pool(name="pst", bufs=4, space="PSUM") as pst,
            tc.tile_pool(name="wups", bufs=1, space="PSUM") as wups,
        ):
            _pe_warmup(nc, wk, wups, n_mm=35)
            Wr_sb = const.tile([128, ET, X], F32)
            nc.sync.dma_start(Wr_sb[:], Wr.rearrange("(a p) x -> p a x", p=128))
            br_sb = const.tile([1, X], F32)
            nc.sync.dma_start(br_sb[:], br[0:1, :])
            ones = const.tile([1, 512], F32)
            nc.vector.memset(ones[:], 1.0)
            id8 = const.tile([8, 8], F32)
            make_identity(nc, id8[:])

            xT_view = xT.rearrange("(a p) t -> p a t", p=128)
            # all input stripes prefetched upfront, per-k-tile granularity so
            # the first matmul starts after 256KB; outputs go on the gpsimd
            # (SWDGE) queue so they never delay input loads
            xts_all = {}
            for nb in range(NB):
                for a in range(ET):
                    t = xtp.tile(
                        [128, 512], F32, tag=f"xts{nb}_{a}", name=f"xts_{nb}_{a}"
                    )
                    nc.sync.dma_start(
                        t[:], xT_view[:, a, nb * 512 : (nb + 1) * 512]
                    )
                    xts_all[nb, a] = t
            for nb in range(NB):
                xts = [xts_all[nb, a] for a in range(ET)]
                ps_t = ps.tile([8, 512], F32, tag="ps")
                for a in range(ET):
                    nc.tensor.matmul(
                        ps_t[:],
                        Wr_sb[:, a, :],
                        xts[a][:],
                        start=(a == 0),
                        stop=False,
                    )
                # bias add as a rank-1 matmul: br[1,8].T @ ones[1,512]
                nc.tensor.matmul(
                    ps_t[:], br_sb[:1, :], ones[:1, :], start=False, stop=True
                )
                lt = wk.tile([8, 512], F32, tag="lt")
                nc.vector.tensor_copy(lt[:], ps_t[:])
                for j in range(4):
                    ps_tr = pst.tile([128, 8], F32, tag="pst")
                    nc.tensor.transpose(
                        ps_tr[:], lt[:, j * 128 : (j + 1) * 128], id8[:]
                    )
                    logits = wk.tile([128, X], F32, tag="logits")
                    nc.vector.tensor_copy(logits[:], ps_tr[:])
                    mx = wk.tile([128, 8], F32, tag="mx")
                    mi = wk.tile([128, 8], U32, tag="mi")
                    nc.vector.max_with_indices(mx[:], mi[:], logits[:])
                    t0 = nb * 512 + j * 128
                    # sync queue: all inputs were prefetched before any
                    # output trigger, and HWDGE drain is ~3us cheaper
                    nc.sync.dma_start(
                        topk[t0 : t0 + 128, :], mi[:, :TOPK].bitcast(I32)
                    )
    nc.compile()
    return nc


def _build_ffn(cap):
    assert cap % 8 == 0
    nc = _make_nc()
    xgT = nc.dram_tensor("xgT", [E, cap], F32R, kind="ExternalInput")
    W1 = nc.dram_tensor("W1", [E, H], F32R, kind="ExternalInput")
    b1 = nc.dram_tensor("b1", [H], F32, kind="ExternalInput")
    W2 = nc.dram_tensor("W2", [H, E], F32R, kind="ExternalInput")
    b2 = nc.dram_tensor("b2", [E], F32, kind="ExternalInput")
    yT = nc.dram_tensor("yT", [E, cap], F32, kind="ExternalOutput")

    # token chunking: near-equal chunks of 128-multiples, each <= 512 and
    # >= 256 so float32r matmuls stay at 1 cycle/row
    nch = -(-cap // 512)
    widths = [512] * nch
    slack = 512 * nch - cap
    i = nch - 1
    while slack > 0:
        d = min(slack, widths[i] - 256) if nch > 1 else slack
        d = (d // 8) * 8
        if d <= 0:
            i = nch - 1 if i == 0 else i - 1
            continue
        widths[i] -= d
        slack -= d
        i = nch - 1 if i == 0 else i - 1
    assert sum(widths) == cap and all(w >= 256 for w in widths), widths
    widths.sort()  # narrowest first: cheapest chunk while weights stream in
    chunks = []
    c0 = 0
    for cw in widths:
        chunks.append((c0, cw))
        c0 += cw

    with tile.TileContext(nc) as tc:
        with (
            tc.tile_pool(name="wconst", bufs=1) as wconst,
            tc.tile_pool(name="xg", bufs=1) as xgp,
            tc.tile_pool(name="g", bufs=2) as gp,
            tc.tile_pool(name="y", bufs=3) as yp,
            tc.tile_pool(name="ps1", bufs=3, space="PSUM") as ps1,
            tc.tile_pool(name="ps2", bufs=3, space="PSUM") as ps2,
            tc.tile_pool(name="wups", bufs=1, space="PSUM") as wups,
        ):
            _pe_warmup(nc, yp, wups)
            # ALL activation chunks ride the gpsimd (SWDGE) DMA queue,
            # emitted upfront, so they are never stuck behind the 12MB of
            # weights on the sync (HWDGE) queue. Chunk 0 is split per k-tile
            # so the first matmul only waits for 1/8 of a chunk.
            xg_view = xgT.rearrange("(a p) t -> p a t", p=128)
            xg0_c0, xg0_cw = chunks[0]
            xg0_parts = [
                wconst.tile([128, xg0_cw], F32R, tag=f"xg0_{a}", name=f"xg0_{a}")
                for a in range(ET)
            ]
            for a in range(ET):
                nc.gpsimd.dma_start(
                    xg0_parts[a][:], xg_view[:, a, xg0_c0 : xg0_c0 + xg0_cw]
                )
            xg_tiles = {}
            for ci, (c0, cw) in enumerate(chunks):
                if ci == 0:
                    continue
                t = xgp.tile([128, ET, cw], F32R, tag="xg", name=f"xg_{ci}")
                nc.gpsimd.dma_start(t[:], xg_view[:, :, c0 : c0 + cw])
                xg_tiles[ci] = t

            b1_sb = wconst.tile([128, HT], F32)
            nc.sync.dma_start(b1_sb[:], b1.rearrange("(h p) -> p h", p=128))
            b2_sb = wconst.tile([128, ET], F32)
            nc.sync.dma_start(b2_sb[:], b2.rearrange("(e p) -> p e", p=128))
            b2h_sb = wconst.tile([128, ET], F32)
            nc.scalar.mul(b2h_sb[:], b2_sb[:], 0.5)

            # weights resident in SBUF, split by hidden tile (DMA'd in the
            # order the PE consumes them: W1 h-tiles, then W2 h-tiles)
            w1_sb = []
            w1_view = W1.rearrange("(a p) (h j) -> h p a j", p=128, j=128)
            for h in range(HT):
                t = wconst.tile([128, ET, 128], F32R, tag=f"w1_{h}")
                nc.sync.dma_start(t[:], w1_view[h])
                w1_sb.append(t)
            w2_sb = []
            w2_view = W2.rearrange("(h p) e -> h p e", p=128)
            for h in range(HT):
                t = wconst.tile([128, E], F32R, tag=f"w2_{h}")
                nc.sync.dma_start(t[:], w2_view[h])
                w2_sb.append(t)

            def phase1(ci):
                c0, cw = chunks[ci]
                if ci == 0:
                    xg_slice = lambda a: xg0_parts[a][:]
                else:
                    xg_sb = xg_tiles[ci]
                    xg_slice = lambda a, xg_sb=xg_sb: xg_sb[:, a, :]
                g_sb = gp.tile([128, HT, cw], F32R, tag="g", name=f"g_{ci}")
                for h in range(HT):
                    pt = ps1.tile([128, cw], F32, tag="ps1", name=f"p1_{ci}_{h}")
                    for a in range(ET):
                        nc.tensor.matmul(
                            pt[:],
                            w1_sb[h][:, a, :],
                            xg_slice(a),
                            start=(a == 0),
                            stop=(a == ET - 1),
                        )
                    nc.scalar.activation(
                        g_sb[:, h, :], pt[:], AF.Gelu, bias=b1_sb[:, h : h + 1]
                    )
                return g_sb

            def phase2(ci, g_sb):
                c0, cw = chunks[ci]
                for eo in range(ET):
                    pt2 = ps2.tile([128, cw], F32, tag="ps2", name=f"p2_{ci}_{eo}")
                    for h in range(HT):
                        nc.tensor.matmul(
                            pt2[:],
                            w2_sb[h][:, eo * 128 : (eo + 1) * 128],
                            g_sb[:, h, :],
                            start=(h == 0),
                            stop=(h == HT - 1),
                        )
                    y_sb = yp.tile([128, cw], F32, tag="y", name=f"y_{ci}_{eo}")
                    # y = 0.5*psum + 0.5*b2  on the (idle) DVE, keeping ACT
                    # free for GELU and avoiding ACT LUT thrash
                    nc.vector.tensor_scalar(
                        y_sb[:],
                        pt2[:],
                        0.5,
                        b2h_sb[:, eo : eo + 1],
                        op0=mybir.AluOpType.mult,
                        op1=mybir.AluOpType.add,
                    )
                    nc.sync.dma_start(
                        yT[eo * 128 : (eo + 1) * 128, c0 : c0 + cw], y_sb[:]
                    )

            for ci in range(len(chunks)):
                phase2(ci, phase1(ci))
    nc.compile()
    return nc


def _get_program(key, builder, *args):
    if key not in _program_cache:
        _program_cache[key] = builder(*args)
    return _program_cache[key]


def _run(nc, in_maps):
    """run_bass_kernel_spmd with one retry for transient runtime errors."""
    try:
        return run_bass_kernel_spmd(nc, in_maps, list(range(NCORES)))
    except Exception:
        return run_bass_kernel_spmd(nc, in_maps, list(range(NCORES)))


def kernel(x, W_router, b_router, W1, b1, W2, b2):
    x = np.ascontiguousarray(np.asarray(x, dtype=np.float32))
    W_router = np.ascontiguousarray(np.asarray(W_router, dtype=np.float32))
    b_router = np.ascontiguousarray(np.asarray(b_router, dtype=np.float32))
    W1 = np.ascontiguousarray(np.asarray(W1, dtype=np.float32))
    b1 = np.ascontiguousarray(np.asarray(b1, dtype=np.float32))
    W2 = np.ascontiguousarray(np.asarray(W2, dtype=np.float32))
    b2 = np.ascontiguousarray(np.asarray(b2, dtype=np.float32))

    x_flat = x.reshape(T, E)
    xT = np.ascontiguousarray(x_flat.T)  # [E, T]
    br2 = b_router.reshape(1, X)

    # ---- launch A: router ----
    ncr = _get_program("router", _build_router)
    in_maps = [
        {"xT": np.ascontiguousarray(xT[:, c * TOK : (c + 1) * TOK]), "Wr": W_router, "br": br2}
        for c in range(NCORES)
    ]
    res_a = _run(ncr, in_maps)
    LAST_RESULTS["router"] = res_a
    topk = np.concatenate(
        [res_a.results[c]["topk"] for c in range(NCORES)], axis=0
    )  # [T, 2] int32

    # ---- host glue: per-expert gather lists ----
    sel = [np.flatnonzero((topk[:, 0] == c) | (topk[:, 1] == c)) for c in range(X)]
    counts = [len(s) for s in sel]
    cap = max(512, -(-max(counts) // 8) * 8)

    ncf = _get_program(("ffn", cap), _build_ffn, cap)
    in_maps = []
    for c in range(X):
        idx = np.zeros(cap, dtype=np.int64)
        idx[: counts[c]] = sel[c]
        xgT = np.ascontiguousarray(x_flat[idx].T)  # [E, cap]
        in_maps.append(
            {"xgT": xgT, "W1": W1[c], "b1": b1[c], "W2": W2[c], "b2": b2[c]}
        )
    res_b = _run(ncf, in_maps)
    LAST_RESULTS["ffn"] = res_b

    # ---- host glue: scatter-add the two 0.5-scaled expert outputs per token ----
    out = np.zeros((T, E), dtype=np.float32)
    for c in range(X):
        yT_c = res_b.results[c]["yT"]  # [E, cap]
        out[sel[c]] += yT_c[:, : counts[c]].T
    return out.reshape(B, N, E), topk.reshape(B, N, TOPK).astype(np.int32)
